# revision 9
# baseline (speedup 1.0000x reference)
"""Trainium2 Bass kernel for nn_AdaGMNConv (gnn_message_passing).

Sharding: one graph per NeuronCore (G=8 graphs, 8 cores). All compute is
local to a core; the host gathers the per-graph scalar outputs.

Per-core math (graph g, M=2048 high-degree nodes per graph, D=128):
  A    = H_g @ F^T                      [2048, 2048]   (fp16 matmul, f32 psum)
  A1   = segment softmax of A over rows (per column)   -> S1 = A1 @ F
  A2   = softmax of A over columns (per row)           -> S2 = A2^T @ H
  out_multi  = MLP([H | S1]); out_single = MLP([F | S2])
  p2 = colsum(out_multi) + colsum(L_g);  p1 = colsum(out_single) + colsum(L_gid)
  out[g] = <p1/||p1||, p2/||p2||>

Key tricks:
  - Both A orientations are computed on TensorE (recompute beats transpose).
  - exp shifts are data-safe precomputed vectors (|f_j|^2 on the gid core via a
    per-core flag tensor, constant 64 elsewhere) -> no max-reduce passes at all;
    exp + denominator accumulation fused in one ScalarE pass over PSUM.
  - Softmax division is folded into the small F/H matmul operands.
  - The MLP's second linear layer collapses onto the pooled vector (only column
    sums of the MLP output are ever needed).
"""

import os
from contextlib import ExitStack

import numpy as np

import concourse.bass as bass
import concourse.tile as tile
from concourse import mybir
from concourse.bass_utils import run_bass_kernel_spmd

f32 = mybir.dt.float32
f16 = mybir.dt.float16
bf16 = mybir.dt.bfloat16

P = 128          # partitions
D = 128          # feature dim
NT = 16          # tiles per 2048-node block
M = P * NT       # 2048 nodes (high-degree per graph == gid block == low block)
SHIFT0 = 64.0    # exp shift for non-gid cores
LN_EPS = 1e-5

MAXW = 1  # walrus in this env rejects >1 sem-wait per instruction


def split_waits(nc, maxw=MAXW):
    """Hoist overflow sem-waits onto preceding same-engine NOPs (this walrus
    build only accepts `maxw` waits per instruction)."""
    ctr = 0
    for fn in nc.m.functions:
        for bb in fn.blocks:
            new_insts = []
            for inst in bb.instructions:
                si = inst.sync_info
                if si is not None and si.on_wait and len(si.on_wait) > maxw:
                    waits = list(si.on_wait)
                    chunks = [waits[i : i + maxw] for i in range(0, len(waits), maxw)]
                    for ch in chunks[:-1]:
                        ctr += 1
                        nop = mybir.InstNoOp(
                            name=f"waitsplit_{ctr}",
                            sync_info=mybir.SyncInfo(on_wait=ch, on_update=[]),
                            bass_nofuse=True,
                            engine=inst.engine,
                        )
                        new_insts.append(nop)
                    si.on_wait = chunks[-1]
                new_insts.append(inst)
            bb.instructions = new_insts
    return ctr


def build_nc(has_b1, has_b2, has_gamma, has_beta):
    nc = bass.Bass()

    # ---- DRAM parameters (per-core shard shapes) ----
    dHTf = nc.declare_dram_parameter("HTf", [D, M], f16, isOutput=False)
    dFTf = nc.declare_dram_parameter("FTf", [D, M], f16, isOutput=False)
    dH = nc.declare_dram_parameter("H", [M, D], f32, isOutput=False)
    dF = nc.declare_dram_parameter("F", [M, D], f32, isOutput=False)
    dL = nc.declare_dram_parameter("L", [M, D], f32, isOutput=False)
    dL0 = nc.declare_dram_parameter("L0", [M, D], f32, isOutput=False)
    dW1 = nc.declare_dram_parameter("W1f", [2 * D, D], f16, isOutput=False)
    dW2 = nc.declare_dram_parameter("W2", [D, D], f32, isOutput=False)
    dISG = nc.declare_dram_parameter("ISG", [P, 1], f32, isOutput=False)
    dB1 = dB2 = dGAM = dBET = None
    if has_b1:
        dB1 = nc.declare_dram_parameter("B1f", [1, D], f16, isOutput=False)
    if has_b2:
        dB2 = nc.declare_dram_parameter("B2", [1, D], f32, isOutput=False)
    if has_gamma:
        dGAM = nc.declare_dram_parameter("GAM", [1, D], f32, isOutput=False)
    if has_beta:
        dBET = nc.declare_dram_parameter("BET", [1, D], f32, isOutput=False)
    dOUT = nc.declare_dram_parameter("out", [1, 1], f32, isOutput=True)

    with tile.TileContext(nc) as tc, ExitStack() as ctx:
        consts = ctx.enter_context(tc.tile_pool(name="consts", bufs=1))
        epool = ctx.enter_context(tc.tile_pool(name="epool", bufs=3))
        scal = ctx.enter_context(tc.tile_pool(name="scal", bufs=4))
        fpp = ctx.enter_context(tc.tile_pool(name="fpp", bufs=3))
        mlpt = ctx.enter_context(tc.tile_pool(name="mlpt", bufs=3))
        psA = ctx.enter_context(tc.tile_pool(name="psA", bufs=2, space="PSUM"))
        psS = ctx.enter_context(tc.tile_pool(name="psS", bufs=1, space="PSUM"))
        psM = ctx.enter_context(tc.tile_pool(name="psM", bufs=2, space="PSUM"))

        # ---- SBUF loads ----
        sb_FTf = consts.tile([P, M], f16)
        sb_HTf = consts.tile([P, M], f16)
        for c in range(4):
            cs = bass.ts(c, M // 4)
            nc.sync.dma_start(out=sb_FTf[:, cs], in_=dFTf[:, cs])
        for c in range(4):
            cs = bass.ts(c, M // 4)
            nc.sync.dma_start(out=sb_HTf[:, cs], in_=dHTf[:, cs])

        sb_F = consts.tile([P, NT, D], f32)
        nc.sync.dma_start(out=sb_F, in_=dF[:, :].rearrange("(t p) d -> p t d", p=P))
        sb_ISG = consts.tile([P, 1], f32)
        nc.sync.dma_start(out=sb_ISG, in_=dISG[:, :])
        sb_W1f = consts.tile([P, 2, D], f16)
        nc.sync.dma_start(out=sb_W1f, in_=dW1[:, :].rearrange("(t p) d -> p t d", p=P))
        sb_H = consts.tile([P, NT, D], f32)
        nc.sync.dma_start(out=sb_H, in_=dH[:, :].rearrange("(t p) d -> p t d", p=P))
        sb_W2 = consts.tile([P, D], f32)
        nc.sync.dma_start(out=sb_W2, in_=dW2[:, :])
        sb_L = consts.tile([P, NT, D], f32)
        nc.sync.dma_start(out=sb_L, in_=dL[:, :].rearrange("(t p) d -> p t d", p=P))
        sb_L0 = consts.tile([P, NT, D], f32)
        nc.sync.dma_start(out=sb_L0, in_=dL0[:, :].rearrange("(t p) d -> p t d", p=P))

        sb_B1f = sb_B2 = None
        if has_b1:
            sb_B1f = consts.tile([1, D], f16)
            nc.sync.dma_start(out=sb_B1f, in_=dB1[:, :])
        if has_b2:
            sb_B2 = consts.tile([1, D], f32)
            nc.sync.dma_start(out=sb_B2, in_=dB2[:, :])
        gam_bc = bet_bc = None
        if has_gamma:
            gam_bc = consts.tile([P, D], f32)
            src = dGAM[:, :]
            nc.sync.dma_start(
                out=gam_bc,
                in_=bass.AP(tensor=src.tensor, offset=src.offset,
                            ap=[[0, P], src.ap[1]]),
            )
        if has_beta:
            bet_bc = consts.tile([P, D], f32)
            src = dBET[:, :]
            nc.sync.dma_start(
                out=bet_bc,
                in_=bass.AP(tensor=src.tensor, offset=src.offset,
                            ap=[[0, P], src.ap[1]]),
            )

        ones_f = consts.tile([P, 1], f32)
        nc.vector.memset(ones_f, 1.0)
        sb_eps = consts.tile([P, 1], f32)
        nc.vector.memset(sb_eps, LN_EPS)
        ones_row = consts.tile([1, D], f16)
        nc.vector.memset(ones_row, 1.0)
        c2048 = consts.tile([1, 1], f32)
        nc.vector.memset(c2048, float(M))

        # ---- negshift[j] = -(isg * |f_j|^2 + (1-isg) * SHIFT0) ----
        sq = consts.tile([P, NT], f32)
        sq_scr = consts.tile([P, D], f32)
        for t in range(NT):
            nc.scalar.activation(
                out=sq_scr,
                in_=sb_F[:, t, :],
                func=mybir.ActivationFunctionType.Square,
                accum_out=sq[:, t : t + 1],
            )
        negshift = consts.tile([P, NT], f32)
        nc.vector.tensor_scalar(
            out=negshift, in0=sq, scalar1=-1.0, scalar2=SHIFT0,
            op0=mybir.AluOpType.mult, op1=mybir.AluOpType.add,
        )  # 64 - sq
        nc.vector.tensor_scalar(
            out=negshift, in0=negshift, scalar1=sb_ISG, scalar2=-SHIFT0,
            op0=mybir.AluOpType.mult, op1=mybir.AluOpType.add,
        )  # isg*(64-sq) - 64

        sb_S1Tf = consts.tile([P, M], f16)
        sb_S2Tf = consts.tile([P, M], f16)

        # ---- Phase 1: A^T tiles -> exp -> den -> F' -> S1T accumulation ----
        # A^T[j, i] with j on partitions: lhsT = FT[:, jtile], rhs = HT.
        ps_s1t = psS.tile([P, M], f32, tag="psS")
        for jt in range(NT):
            jr = bass.ts(jt, P)
            et = epool.tile([P, M], bf16, tag="E")
            dparts = scal.tile([P, 4], f32, tag="dparts")
            for c in range(4):
                cs = bass.ts(c, 512)
                pa = psA.tile([P, 512], f32, tag="psA")
                nc.tensor.matmul(pa, lhsT=sb_FTf[:, jr], rhs=sb_HTf[:, cs],
                                 start=True, stop=True)
                nc.scalar.activation(
                    out=et[:, cs], in_=pa,
                    func=mybir.ActivationFunctionType.Exp,
                    bias=negshift[:, jt : jt + 1], scale=1.0,
                    accum_out=dparts[:, c : c + 1],
                )
            den = scal.tile([P, 1], f32, tag="den")
            nc.vector.reduce_sum(out=den, in_=dparts, axis=mybir.AxisListType.X)
            rec = scal.tile([P, 1], f32, tag="rec")
            nc.vector.reciprocal(out=rec, in_=den)
            fp = fpp.tile([P, D], bf16, tag="fp")
            nc.vector.tensor_scalar_mul(out=fp, in0=sb_F[:, jt, :], scalar1=rec)
            for c in range(4):
                cs = bass.ts(c, 512)
                nc.tensor.matmul(
                    ps_s1t[:, cs], lhsT=fp, rhs=et[:, cs],
                    start=(jt == 0), stop=(jt == NT - 1),
                )
        nc.vector.tensor_copy(out=sb_S1Tf, in_=ps_s1t)

        # ---- Phase 2: A tiles -> exp -> den -> H' -> S2T accumulation ----
        ps_s2t = psS.tile([P, M], f32, tag="psS")
        for it in range(NT):
            ir = bass.ts(it, P)
            et = epool.tile([P, M], bf16, tag="E")
            dparts = scal.tile([P, 4], f32, tag="dparts")
            for c in range(4):
                cs = bass.ts(c, 512)
                pa = psA.tile([P, 512], f32, tag="psA")
                nc.tensor.matmul(pa, lhsT=sb_HTf[:, ir], rhs=sb_FTf[:, cs],
                                 start=True, stop=True)
                nc.scalar.activation(
                    out=et[:, cs], in_=pa,
                    func=mybir.ActivationFunctionType.Exp,
                    bias=negshift[:, it : it + 1], scale=1.0,
                    accum_out=dparts[:, c : c + 1],
                )
            den = scal.tile([P, 1], f32, tag="den")
            nc.vector.reduce_sum(out=den, in_=dparts, axis=mybir.AxisListType.X)
            rec = scal.tile([P, 1], f32, tag="rec")
            nc.vector.reciprocal(out=rec, in_=den)
            hp = fpp.tile([P, D], bf16, tag="fp")
            nc.vector.tensor_scalar_mul(out=hp, in0=sb_H[:, it, :], scalar1=rec)
            for c in range(4):
                cs = bass.ts(c, 512)
                nc.tensor.matmul(
                    ps_s2t[:, cs], lhsT=hp, rhs=et[:, cs],
                    start=(it == 0), stop=(it == NT - 1),
                )
        nc.vector.tensor_copy(out=sb_S2Tf, in_=ps_s2t)

        # ---- MLP phases: pre-act -> LayerNorm -> ReLU -> column-sum ----
        def mlp_colsum(xT_f16, sT_f16, racc, r_sb):
            """r_sb[d,1] = sum over nodes of relu(LN([x|s] @ W1 + b1)) with
            xT/sT the feature-major [128, 2048] fp16 halves of the input.
            relu tiles are accumulated on DVE into racc, then one
            ones-matmul folds the partition axis."""
            for t in range(NT):
                tr = bass.ts(t, P)
                pre = psM.tile([P, D], f32, tag="pre")
                nc.tensor.matmul(pre, lhsT=xT_f16[:, tr], rhs=sb_W1f[:, 0, :],
                                 start=True, stop=False)
                nc.tensor.matmul(pre, lhsT=sT_f16[:, tr], rhs=sb_W1f[:, 1, :],
                                 start=False, stop=not has_b1)
                if has_b1:
                    nc.tensor.matmul(pre, lhsT=ones_row, rhs=sb_B1f,
                                     start=False, stop=True)
                stats = scal.tile([P, 6], f32, tag="stats")
                nc.vector.bn_stats(out=stats, in_=pre)
                mv = scal.tile([P, 2], f32, tag="mv")
                nc.vector.bn_aggr(out=mv, in_=stats)
                stdv = scal.tile([P, 1], f32, tag="stdv")
                nc.scalar.activation(out=stdv, in_=mv[:, 1:2],
                                     func=mybir.ActivationFunctionType.Sqrt,
                                     bias=sb_eps, scale=1.0)
                rstd = scal.tile([P, 1], f32, tag="rstd")
                nc.vector.reciprocal(out=rstd, in_=stdv)
                tt = mlpt.tile([P, D], f32, tag="tt")
                nc.vector.tensor_scalar(
                    out=tt, in0=pre, scalar1=mv[:, 0:1], scalar2=rstd,
                    op0=mybir.AluOpType.subtract, op1=mybir.AluOpType.mult,
                )
                if has_gamma:
                    nc.vector.tensor_mul(out=tt, in0=tt, in1=gam_bc)
                if has_beta:
                    nc.vector.tensor_add(out=tt, in0=tt, in1=bet_bc)
                rl = mlpt.tile([P, D], f32, tag="rl")
                nc.scalar.activation(out=rl, in_=tt,
                                     func=mybir.ActivationFunctionType.Relu)
                if t == 0:
                    nc.vector.tensor_copy(out=racc, in_=rl)
                else:
                    nc.vector.tensor_add(out=racc, in0=racc, in1=rl)
            ps_r = psM.tile([P, 1], f32, tag="pre")
            nc.tensor.matmul(ps_r, lhsT=racc, rhs=ones_f, start=True, stop=True)
            nc.vector.tensor_copy(out=r_sb, in_=ps_r)

        racc2 = consts.tile([P, D], f32)
        r2_sb = consts.tile([P, 1], f32)
        mlp_colsum(sb_HTf, sb_S1Tf, racc2, r2_sb)

        racc1 = consts.tile([P, D], f32)
        r1_sb = consts.tile([P, 1], f32)
        mlp_colsum(sb_FTf, sb_S2Tf, racc1, r1_sb)

        # ---- pooled vectors p2 = W2^T r2 + colsum(L) + M*b2 (transposed) ----
        pcat = consts.tile([P, 2], f32)

        ps_p2 = psM.tile([P, 1], f32, tag="pre")
        nc.tensor.matmul(ps_p2, lhsT=sb_W2, rhs=r2_sb, start=True, stop=False)
        for t in range(NT):
            last = (t == NT - 1) and not has_b2
            nc.tensor.matmul(ps_p2, lhsT=sb_L[:, t, :], rhs=ones_f,
                             start=False, stop=last)
        if has_b2:
            nc.tensor.matmul(ps_p2, lhsT=sb_B2, rhs=c2048, start=False, stop=True)
        nc.vector.tensor_copy(out=pcat[:, 1:2], in_=ps_p2)

        ps_p1 = psM.tile([P, 1], f32, tag="pre")
        nc.tensor.matmul(ps_p1, lhsT=sb_W2, rhs=r1_sb, start=True, stop=False)
        for t in range(NT):
            last = (t == NT - 1) and not has_b2
            nc.tensor.matmul(ps_p1, lhsT=sb_L0[:, t, :], rhs=ones_f,
                             start=False, stop=last)
        if has_b2:
            nc.tensor.matmul(ps_p1, lhsT=sb_B2, rhs=c2048, start=False, stop=True)
        nc.vector.tensor_copy(out=pcat[:, 0:1], in_=ps_p1)

        # ---- final: out = <p1, p2> / (max(||p1||,eps) * max(||p2||,eps)) ----
        ps_d1 = psM.tile([1, 2], f32, tag="pre")
        nc.tensor.matmul(ps_d1, lhsT=pcat[:, 0:1], rhs=pcat, start=True, stop=True)
        ps_d2 = psM.tile([1, 1], f32, tag="pre")
        nc.tensor.matmul(ps_d2, lhsT=pcat[:, 1:2], rhs=pcat[:, 1:2],
                         start=True, stop=True)
        dots = consts.tile([1, 4], f32)
        nc.vector.tensor_copy(out=dots[:, 0:2], in_=ps_d1)   # s11, s12
        nc.vector.tensor_copy(out=dots[:, 2:3], in_=ps_d2)   # s22
        q = consts.tile([1, 1], f32)
        nc.vector.tensor_mul(out=q, in0=dots[:, 0:1], in1=dots[:, 2:3])
        nc.scalar.activation(out=q, in_=q,
                             func=mybir.ActivationFunctionType.Sqrt,
                             bias=0.0, scale=1.0)
        nc.vector.tensor_scalar_max(out=q, in0=q, scalar1=1e-24)
        rq = consts.tile([1, 1], f32)
        nc.vector.reciprocal(out=rq, in_=q)
        res = consts.tile([1, 1], f32)
        nc.vector.tensor_mul(out=res, in0=dots[:, 1:2], in1=rq)
        nc.sync.dma_start(out=dOUT[:, :], in_=res)

    split_waits(nc)
    return nc


_BUILD_CACHE = {}


def _get_nc(flags):
    if flags not in _BUILD_CACHE:
        _BUILD_CACHE[flags] = build_nc(*flags)
    return _BUILD_CACHE[flags]


def kernel(x, edge_attr, W1, b1, gamma, beta, W2, b2, gid, edge_index, batch):
    x = np.asarray(x, dtype=np.float32)
    W1 = np.asarray(W1, dtype=np.float32)
    b1 = np.asarray(b1, dtype=np.float32)
    gamma = np.asarray(gamma, dtype=np.float32)
    beta = np.asarray(beta, dtype=np.float32)
    W2 = np.asarray(W2, dtype=np.float32)
    b2 = np.asarray(b2, dtype=np.float32)
    gid = int(np.asarray(gid))
    ei0 = np.asarray(edge_index)[0]
    b = np.asarray(batch)

    N, Dx = x.shape
    assert Dx == D
    deg = np.bincount(ei0, minlength=N)
    mask = deg > 1
    G = int(b.max()) + 1
    assert G == 8
    hd_idx = np.where(mask)[0]
    fhb = b[hd_idx]
    Mtot = hd_idx.size
    assert Mtot % G == 0 and np.array_equal(
        fhb, np.repeat(np.arange(G), Mtot // G)
    )
    assert Mtot // G == M

    gxf_idx = np.where(mask & (b == gid))[0]
    assert gxf_idx.size == M
    F = np.ascontiguousarray(x[gxf_idx])
    FTf = np.ascontiguousarray(F.T).astype(np.float16)
    lo0_idx = np.where((~mask) & (b == gid))[0]
    assert lo0_idx.size == M
    L0 = np.ascontiguousarray(x[lo0_idx])

    flags = (
        bool(np.any(b1 != 0.0)),
        bool(np.any(b2 != 0.0)),
        bool(np.any(gamma != 1.0)),
        bool(np.any(beta != 0.0)),
    )
    has_b1, has_b2, has_gamma, has_beta = flags
    nc = _get_nc(flags)

    W1f = W1.astype(np.float16)
    in_maps = []
    for g in range(G):
        sel_h = mask & (b == g)
        sel_l = (~mask) & (b == g)
        assert sel_h.sum() == M and sel_l.sum() == M
        H = np.ascontiguousarray(x[sel_h])
        L = np.ascontiguousarray(x[sel_l])
        im = {
            "HTf": np.ascontiguousarray(H.T).astype(np.float16),
            "FTf": FTf,
            "H": H,
            "F": F,
            "L": L,
            "L0": L0,
            "W1f": W1f,
            "W2": W2,
            "ISG": np.full((P, 1), 1.0 if g == gid else 0.0, np.float32),
        }
        if has_b1:
            im["B1f"] = b1.reshape(1, D).astype(np.float16)
        if has_b2:
            im["B2"] = b2.reshape(1, D).astype(np.float32)
        if has_gamma:
            im["GAM"] = gamma.reshape(1, D)
        if has_beta:
            im["BET"] = beta.reshape(1, D)
        in_maps.append(im)

    trace_dir = os.environ.get("ADAGMN_TRACE", "")
    if trace_dir:
        res = run_bass_kernel_spmd(
            nc, in_maps, core_ids=list(range(G)), trace=True, tmpdir=trace_dir
        )
        print(f"HW exec time: {res.exec_time_ns} ns")
    else:
        res = run_bass_kernel_spmd(nc, in_maps, core_ids=list(range(G)))
    out = np.array([res.results[g]["out"][0, 0] for g in range(G)], np.float32)
    return out


# revision 17
# speedup vs baseline: 1.3715x; 1.3715x over previous
"""Trainium2 Bass kernel for nn_AdaGMNConv (gnn_message_passing).

Sharding: one graph per NeuronCore (G=8 graphs, 8 cores). All compute is
local to a core; the host gathers the per-graph scalar outputs.

Per-core math (graph g, M=2048 high-degree nodes per graph, D=128):
  A    = H_g @ F^T                      [2048, 2048]   (fp16 matmul, f32 psum)
  A1   = segment softmax of A over rows (per column)   -> S1 = A1 @ F
  A2   = softmax of A over columns (per row)           -> S2 = A2^T @ H
  out_multi  = MLP([H | S1]); out_single = MLP([F | S2])
  p2 = colsum(out_multi) + colsum(L_g);  p1 = colsum(out_single) + colsum(L_gid)
  out[g] = <p1/||p1||, p2/||p2||>

Key tricks:
  - Both A orientations are computed on TensorE (recompute beats transpose).
  - exp shifts are data-safe precomputed vectors (|f_j|^2 on the gid core via a
    per-core flag tensor, constant 64 elsewhere) -> no max-reduce passes at all;
    exp + denominator accumulation fused in one ScalarE pass over PSUM.
  - Softmax division is folded into the small F/H matmul operands.
  - The MLP's second linear layer collapses onto the pooled vector (only column
    sums of the MLP output are ever needed).
"""

import os
from contextlib import ExitStack

import numpy as np

import concourse.bass as bass
import concourse.tile as tile
from concourse import mybir
from concourse.bass_utils import run_bass_kernel_spmd

f32 = mybir.dt.float32
f16 = mybir.dt.float16
bf16 = mybir.dt.bfloat16

P = 128          # partitions
D = 128          # feature dim
NT = 16          # tiles per 2048-node block
M = P * NT       # 2048 nodes (high-degree per graph == gid block == low block)
SHIFT0 = 64.0    # exp shift for non-gid cores
LN_EPS = 1e-5
CH = 1024        # PSUM chunk width for the attention tiles (2 banks)
MMN = 512        # matmul moving free-dim (one PSUM bank)

MAXW = 1  # walrus in this env rejects >1 sem-wait per instruction


def split_waits(nc, maxw=MAXW):
    """Hoist overflow sem-waits onto preceding same-engine NOPs (this walrus
    build only accepts `maxw` waits per instruction)."""
    ctr = 0
    for fn in nc.m.functions:
        for bb in fn.blocks:
            new_insts = []
            for inst in bb.instructions:
                si = inst.sync_info
                if si is not None and si.on_wait and len(si.on_wait) > maxw:
                    waits = list(si.on_wait)
                    chunks = [waits[i : i + maxw] for i in range(0, len(waits), maxw)]
                    for ch in chunks[:-1]:
                        ctr += 1
                        nop = mybir.InstNoOp(
                            name=f"waitsplit_{ctr}",
                            sync_info=mybir.SyncInfo(on_wait=ch, on_update=[]),
                            bass_nofuse=True,
                            engine=inst.engine,
                        )
                        new_insts.append(nop)
                    si.on_wait = chunks[-1]
                new_insts.append(inst)
            bb.instructions = new_insts
    return ctr


def build_nc(has_b1, has_b2, has_gamma, has_beta):
    nc = bass.Bass()

    # ---- DRAM parameters (per-core shard shapes) ----
    dHTf = nc.declare_dram_parameter("HTf", [D, M], f16, isOutput=False)
    dFTf = nc.declare_dram_parameter("FTf", [D, M], f16, isOutput=False)
    dH = nc.declare_dram_parameter("H", [M, D], f32, isOutput=False)
    dF = nc.declare_dram_parameter("F", [M, D], f32, isOutput=False)
    dL = nc.declare_dram_parameter("L", [M, D], f32, isOutput=False)
    dL0 = nc.declare_dram_parameter("L0", [M, D], f32, isOutput=False)
    dW1 = nc.declare_dram_parameter("W1f", [2 * D, D], f16, isOutput=False)
    dW2 = nc.declare_dram_parameter("W2", [D, D], f32, isOutput=False)
    dISG = nc.declare_dram_parameter("ISG", [P, 1], f32, isOutput=False)
    dB1 = dB2 = dGAM = dBET = None
    if has_b1:
        dB1 = nc.declare_dram_parameter("B1f", [1, D], f16, isOutput=False)
    if has_b2:
        dB2 = nc.declare_dram_parameter("B2", [1, D], f32, isOutput=False)
    if has_gamma:
        dGAM = nc.declare_dram_parameter("GAM", [1, D], f32, isOutput=False)
    if has_beta:
        dBET = nc.declare_dram_parameter("BET", [1, D], f32, isOutput=False)
    dOUT = nc.declare_dram_parameter("out", [1, 1], f32, isOutput=True)

    with tile.TileContext(nc) as tc, ExitStack() as ctx:
        consts = ctx.enter_context(tc.tile_pool(name="consts", bufs=1))
        epool = ctx.enter_context(tc.tile_pool(name="epool", bufs=3))
        scal = ctx.enter_context(tc.tile_pool(name="scal", bufs=4))
        fpp = ctx.enter_context(tc.tile_pool(name="fpp", bufs=3))
        mlpt = ctx.enter_context(tc.tile_pool(name="mlpt", bufs=3))
        # PSUM budget (8 banks): psA = 2 slots x [128,1024] (4 banks) shared by
        # the A-chunk tiles, MLP pre-act tiles and the small tail matmuls;
        # psS = 1 slot x [128,2048] (4 banks) for the S1T/S2T accumulators.
        psA = ctx.enter_context(tc.tile_pool(name="psA", bufs=2, space="PSUM"))
        psS = ctx.enter_context(tc.tile_pool(name="psS", bufs=1, space="PSUM"))

        # ---- SBUF loads ----
        sb_FTf = consts.tile([P, M], f16)
        sb_HTf = consts.tile([P, M], f16)
        for c in range(4):
            cs = bass.ts(c, M // 4)
            nc.sync.dma_start(out=sb_FTf[:, cs], in_=dFTf[:, cs])
        for c in range(4):
            cs = bass.ts(c, M // 4)
            nc.sync.dma_start(out=sb_HTf[:, cs], in_=dHTf[:, cs])

        sb_F = consts.tile([P, NT, D], f32)
        nc.sync.dma_start(out=sb_F, in_=dF[:, :].rearrange("(t p) d -> p t d", p=P))
        sb_ISG = consts.tile([P, 1], f32)
        nc.sync.dma_start(out=sb_ISG, in_=dISG[:, :])
        sb_W1f = consts.tile([P, 2, D], f16)
        nc.sync.dma_start(out=sb_W1f, in_=dW1[:, :].rearrange("(t p) d -> p t d", p=P))
        sb_H = consts.tile([P, NT, D], f32)
        nc.sync.dma_start(out=sb_H, in_=dH[:, :].rearrange("(t p) d -> p t d", p=P))
        sb_W2 = consts.tile([P, D], f32)
        nc.sync.dma_start(out=sb_W2, in_=dW2[:, :])
        sb_L = consts.tile([P, NT, D], f32)
        nc.sync.dma_start(out=sb_L, in_=dL[:, :].rearrange("(t p) d -> p t d", p=P))
        sb_L0 = consts.tile([P, NT, D], f32)
        nc.sync.dma_start(out=sb_L0, in_=dL0[:, :].rearrange("(t p) d -> p t d", p=P))

        sb_B1f = sb_B2 = None
        if has_b1:
            sb_B1f = consts.tile([1, D], f16)
            nc.sync.dma_start(out=sb_B1f, in_=dB1[:, :])
        if has_b2:
            sb_B2 = consts.tile([1, D], f32)
            nc.sync.dma_start(out=sb_B2, in_=dB2[:, :])
        gam_bc = bet_bc = None
        if has_gamma:
            gam_bc = consts.tile([P, D], f32)
            src = dGAM[:, :]
            nc.sync.dma_start(
                out=gam_bc,
                in_=bass.AP(tensor=src.tensor, offset=src.offset,
                            ap=[[0, P], src.ap[1]]),
            )
        if has_beta:
            bet_bc = consts.tile([P, D], f32)
            src = dBET[:, :]
            nc.sync.dma_start(
                out=bet_bc,
                in_=bass.AP(tensor=src.tensor, offset=src.offset,
                            ap=[[0, P], src.ap[1]]),
            )

        ones_f = consts.tile([P, 1], f32)
        nc.vector.memset(ones_f, 1.0)
        sb_eps = consts.tile([P, 1], f32)
        nc.vector.memset(sb_eps, LN_EPS)
        ones_row = consts.tile([1, D], f16)
        nc.vector.memset(ones_row, 1.0)
        c2048 = consts.tile([1, 1], f32)
        nc.vector.memset(c2048, float(M))

        # ---- negshift[j] = -(isg * |f_j|^2 + (1-isg) * SHIFT0) ----
        sq = consts.tile([P, NT], f32)
        sq_scr = consts.tile([P, D], f32)
        for t in range(NT):
            nc.scalar.activation(
                out=sq_scr,
                in_=sb_F[:, t, :],
                func=mybir.ActivationFunctionType.Square,
                accum_out=sq[:, t : t + 1],
            )
        negshift = consts.tile([P, NT], f32)
        nc.vector.tensor_scalar(
            out=negshift, in0=sq, scalar1=-1.0, scalar2=SHIFT0,
            op0=mybir.AluOpType.mult, op1=mybir.AluOpType.add,
        )  # 64 - sq
        nc.vector.tensor_scalar(
            out=negshift, in0=negshift, scalar1=sb_ISG, scalar2=-SHIFT0,
            op0=mybir.AluOpType.mult, op1=mybir.AluOpType.add,
        )  # isg*(64-sq) - 64

        sb_S1Tf = consts.tile([P, M], f16)
        sb_S2Tf = consts.tile([P, M], f16)

        # ---- Phases 1/2: A tiles -> exp -> den -> scaled operand -> S accum ----
        # Phase 1: A^T[j, i] (j on partitions): lhsT = FT[:, jt], rhs = HT;
        #          S1T[d, i] += F'[jt]^T @ E1[jt].
        # Phase 2: A[i, j]: lhsT = HT[:, it], rhs = FT; S2T[d, j] += H'.T @ E2.
        NCH = M // CH

        def softmax_phase(statT, movT, scaled_src, ps_acc):
            for t in range(NT):
                tr = bass.ts(t, P)
                et = epool.tile([P, M], bf16, tag="E")
                dparts = scal.tile([P, NCH], f32, tag="dparts")
                for c in range(NCH):
                    pa = psA.tile([P, CH], f32, tag="psA")
                    for h in range(CH // MMN):
                        hs = bass.ts(h, MMN)
                        nc.tensor.matmul(
                            pa[:, hs], lhsT=statT[:, tr],
                            rhs=movT[:, bass.ds(c * CH + h * MMN, MMN)],
                            start=True, stop=True,
                        )
                    nc.scalar.activation(
                        out=et[:, bass.ts(c, CH)], in_=pa,
                        func=mybir.ActivationFunctionType.Exp,
                        bias=negshift[:, t : t + 1], scale=1.0,
                        accum_out=dparts[:, c : c + 1],
                    )
                den = scal.tile([P, 1], f32, tag="den")
                nc.vector.reduce_sum(out=den, in_=dparts, axis=mybir.AxisListType.X)
                rec = scal.tile([P, 1], f32, tag="rec")
                nc.vector.reciprocal(out=rec, in_=den)
                fp = fpp.tile([P, D], bf16, tag="fp")
                nc.vector.tensor_scalar_mul(out=fp, in0=scaled_src[:, t, :],
                                            scalar1=rec)
                for h in range(M // MMN):
                    hs = bass.ts(h, MMN)
                    nc.tensor.matmul(
                        ps_acc[:, hs], lhsT=fp, rhs=et[:, hs],
                        start=(t == 0), stop=(t == NT - 1),
                    )

        ps_s1t = psS.tile([P, M], f32, tag="psS")
        softmax_phase(sb_FTf, sb_HTf, sb_F, ps_s1t)
        nc.vector.tensor_copy(out=sb_S1Tf, in_=ps_s1t)

        ps_s2t = psS.tile([P, M], f32, tag="psS")
        softmax_phase(sb_HTf, sb_FTf, sb_H, ps_s2t)
        nc.vector.tensor_copy(out=sb_S2Tf, in_=ps_s2t)

        # ---- MLP phases: pre-act -> LayerNorm -> ReLU -> column-sum ----
        # Two-stage: (1) all pre-act tiles to SBUF + bn stats, (2) one batched
        # rstd = exp(-0.5*ln(var+eps)) (same ACT table set as Exp), then
        # normalize + fused relu-accumulate on DVE.
        def mlp_colsum(xT_f16, sT_f16, pre_all, mvall, racc, r_sb):
            """r_sb[d,1] = sum over nodes of relu(LN([x|s] @ W1 + b1)) with
            xT/sT the feature-major [128, 2048] fp16 halves of the input."""
            for t in range(NT):
                tr = bass.ts(t, P)
                pre = psA.tile([P, D], f32, tag="psA")
                nc.tensor.matmul(pre, lhsT=xT_f16[:, tr], rhs=sb_W1f[:, 0, :],
                                 start=True, stop=False)
                nc.tensor.matmul(pre, lhsT=sT_f16[:, tr], rhs=sb_W1f[:, 1, :],
                                 start=False, stop=not has_b1)
                if has_b1:
                    nc.tensor.matmul(pre, lhsT=ones_row, rhs=sb_B1f,
                                     start=False, stop=True)
                nc.vector.tensor_copy(out=pre_all[:, t, :], in_=pre)
                stats = scal.tile([P, 6], f32, tag="stats")
                nc.vector.bn_stats(out=stats, in_=pre_all[:, t, :])
                nc.vector.bn_aggr(out=mvall[:, t, :], in_=stats)
            lnv = scal.tile([P, NT], f32, tag="lnv")
            nc.scalar.activation(out=lnv, in_=mvall[:, :, 1],
                                 func=mybir.ActivationFunctionType.Ln,
                                 bias=sb_eps, scale=1.0)
            rstd_all = consts.tile([P, NT], f32)
            nc.scalar.activation(out=rstd_all, in_=lnv,
                                 func=mybir.ActivationFunctionType.Exp,
                                 scale=-0.5)
            for t in range(NT):
                tt = mlpt.tile([P, D], f32, tag="tt")
                nc.vector.tensor_scalar(
                    out=tt, in0=pre_all[:, t, :], scalar1=mvall[:, t, 0:1],
                    scalar2=rstd_all[:, t : t + 1],
                    op0=mybir.AluOpType.subtract, op1=mybir.AluOpType.mult,
                )
                if has_gamma:
                    nc.vector.tensor_mul(out=tt, in0=tt, in1=gam_bc)
                if has_beta:
                    nc.vector.tensor_add(out=tt, in0=tt, in1=bet_bc)
                if t == 0:
                    nc.vector.tensor_scalar_max(out=racc, in0=tt, scalar1=0.0)
                else:
                    nc.vector.scalar_tensor_tensor(
                        out=racc, in0=tt, scalar=0.0, in1=racc,
                        op0=mybir.AluOpType.max, op1=mybir.AluOpType.add,
                    )
            ps_r = psA.tile([P, 1], f32, tag="psA")
            nc.tensor.matmul(ps_r, lhsT=racc, rhs=ones_f, start=True, stop=True)
            nc.vector.tensor_copy(out=r_sb, in_=ps_r)

        pre_all2 = consts.tile([P, NT, D], f32)
        mvall2 = consts.tile([P, NT, 2], f32)
        racc2 = consts.tile([P, D], f32)
        r2_sb = consts.tile([P, 1], f32)
        mlp_colsum(sb_HTf, sb_S1Tf, pre_all2, mvall2, racc2, r2_sb)

        pre_all1 = consts.tile([P, NT, D], f32)
        mvall1 = consts.tile([P, NT, 2], f32)
        racc1 = consts.tile([P, D], f32)
        r1_sb = consts.tile([P, 1], f32)
        mlp_colsum(sb_FTf, sb_S2Tf, pre_all1, mvall1, racc1, r1_sb)

        # ---- pooled vectors p2 = W2^T r2 + colsum(L) + M*b2 (transposed) ----
        pcat = consts.tile([P, 2], f32)

        ps_p2 = psA.tile([P, 1], f32, tag="psA")
        nc.tensor.matmul(ps_p2, lhsT=sb_W2, rhs=r2_sb, start=True, stop=False)
        for t in range(NT):
            last = (t == NT - 1) and not has_b2
            nc.tensor.matmul(ps_p2, lhsT=sb_L[:, t, :], rhs=ones_f,
                             start=False, stop=last)
        if has_b2:
            nc.tensor.matmul(ps_p2, lhsT=sb_B2, rhs=c2048, start=False, stop=True)
        nc.vector.tensor_copy(out=pcat[:, 1:2], in_=ps_p2)

        ps_p1 = psA.tile([P, 1], f32, tag="psA")
        nc.tensor.matmul(ps_p1, lhsT=sb_W2, rhs=r1_sb, start=True, stop=False)
        for t in range(NT):
            last = (t == NT - 1) and not has_b2
            nc.tensor.matmul(ps_p1, lhsT=sb_L0[:, t, :], rhs=ones_f,
                             start=False, stop=last)
        if has_b2:
            nc.tensor.matmul(ps_p1, lhsT=sb_B2, rhs=c2048, start=False, stop=True)
        nc.vector.tensor_copy(out=pcat[:, 0:1], in_=ps_p1)

        # ---- final: out = <p1, p2> / (max(||p1||,eps) * max(||p2||,eps)) ----
        ps_d1 = psA.tile([1, 2], f32, tag="psA")
        nc.tensor.matmul(ps_d1, lhsT=pcat[:, 0:1], rhs=pcat, start=True, stop=True)
        ps_d2 = psA.tile([1, 1], f32, tag="psA")
        nc.tensor.matmul(ps_d2, lhsT=pcat[:, 1:2], rhs=pcat[:, 1:2],
                         start=True, stop=True)
        dots = consts.tile([1, 4], f32)
        nc.vector.tensor_copy(out=dots[:, 0:2], in_=ps_d1)   # s11, s12
        nc.vector.tensor_copy(out=dots[:, 2:3], in_=ps_d2)   # s22
        # out = s12 * (s11*s22)^-0.5 via exp(-0.5*ln(q)) — same ACT table set.
        q = consts.tile([1, 1], f32)
        nc.vector.tensor_mul(out=q, in0=dots[:, 0:1], in1=dots[:, 2:3])
        nc.vector.tensor_scalar_max(out=q, in0=q, scalar1=1e-30)
        lq = consts.tile([1, 1], f32)
        nc.scalar.activation(out=lq, in_=q,
                             func=mybir.ActivationFunctionType.Ln)
        rq = consts.tile([1, 1], f32)
        nc.scalar.activation(out=rq, in_=lq,
                             func=mybir.ActivationFunctionType.Exp,
                             scale=-0.5)
        res = consts.tile([1, 1], f32)
        nc.vector.tensor_mul(out=res, in0=dots[:, 1:2], in1=rq)
        nc.sync.dma_start(out=dOUT[:, :], in_=res)

    split_waits(nc)
    return nc


_BUILD_CACHE = {}


def _get_nc(flags):
    if flags not in _BUILD_CACHE:
        _BUILD_CACHE[flags] = build_nc(*flags)
    return _BUILD_CACHE[flags]


def kernel(x, edge_attr, W1, b1, gamma, beta, W2, b2, gid, edge_index, batch):
    x = np.asarray(x, dtype=np.float32)
    W1 = np.asarray(W1, dtype=np.float32)
    b1 = np.asarray(b1, dtype=np.float32)
    gamma = np.asarray(gamma, dtype=np.float32)
    beta = np.asarray(beta, dtype=np.float32)
    W2 = np.asarray(W2, dtype=np.float32)
    b2 = np.asarray(b2, dtype=np.float32)
    gid = int(np.asarray(gid))
    ei0 = np.asarray(edge_index)[0]
    b = np.asarray(batch)

    N, Dx = x.shape
    assert Dx == D
    deg = np.bincount(ei0, minlength=N)
    mask = deg > 1
    G = int(b.max()) + 1
    assert G == 8
    hd_idx = np.where(mask)[0]
    fhb = b[hd_idx]
    Mtot = hd_idx.size
    assert Mtot % G == 0 and np.array_equal(
        fhb, np.repeat(np.arange(G), Mtot // G)
    )
    assert Mtot // G == M

    gxf_idx = np.where(mask & (b == gid))[0]
    assert gxf_idx.size == M
    F = np.ascontiguousarray(x[gxf_idx])
    FTf = np.ascontiguousarray(F.T).astype(np.float16)
    lo0_idx = np.where((~mask) & (b == gid))[0]
    assert lo0_idx.size == M
    L0 = np.ascontiguousarray(x[lo0_idx])

    flags = (
        bool(np.any(b1 != 0.0)),
        bool(np.any(b2 != 0.0)),
        bool(np.any(gamma != 1.0)),
        bool(np.any(beta != 0.0)),
    )
    has_b1, has_b2, has_gamma, has_beta = flags
    nc = _get_nc(flags)

    W1f = W1.astype(np.float16)
    in_maps = []
    for g in range(G):
        sel_h = mask & (b == g)
        sel_l = (~mask) & (b == g)
        assert sel_h.sum() == M and sel_l.sum() == M
        H = np.ascontiguousarray(x[sel_h])
        L = np.ascontiguousarray(x[sel_l])
        im = {
            "HTf": np.ascontiguousarray(H.T).astype(np.float16),
            "FTf": FTf,
            "H": H,
            "F": F,
            "L": L,
            "L0": L0,
            "W1f": W1f,
            "W2": W2,
            "ISG": np.full((P, 1), 1.0 if g == gid else 0.0, np.float32),
        }
        if has_b1:
            im["B1f"] = b1.reshape(1, D).astype(np.float16)
        if has_b2:
            im["B2"] = b2.reshape(1, D).astype(np.float32)
        if has_gamma:
            im["GAM"] = gamma.reshape(1, D)
        if has_beta:
            im["BET"] = beta.reshape(1, D)
        in_maps.append(im)

    trace_dir = os.environ.get("ADAGMN_TRACE", "")
    if trace_dir:
        res = run_bass_kernel_spmd(
            nc, in_maps, core_ids=list(range(G)), trace=True, tmpdir=trace_dir
        )
        print(f"HW exec time: {res.exec_time_ns} ns")
    else:
        res = run_bass_kernel_spmd(nc, in_maps, core_ids=list(range(G)))
    out = np.array([res.results[g]["out"][0, 0] for g in range(G)], np.float32)
    return out
